# revision 18
# baseline (speedup 1.0000x reference)
"""Trainium2 Bass kernel for nn_DistAttn (GNN edge-softmax message passing).

Strategy (8 NeuronCores, SPMD single program), "design V":
  - Destination-node sharding: nodes packed into 320 bins (8 cores x 40
    blocks of <=128 dst slots) by a degree-balancing greedy; every edge
    lives on exactly one core; per-block edge counts near-uniform.
  - Score refactor: e = f_dst (Wq Wk^T) f_src^T, so K rows are the RAW
    bf16 features (host-supplied table featR, available at t=0) and the
    per-block query tiles are QT[c,d] = B^T @ feat_blk^T with the tiny
    host-computed B = Wq @ Wk^T.  Only the h table (feat @ W_fc) is
    built on device (phase 1), as bf16 DRAM rows split low/high at
    row 22528 so gather indices fit int16 and the low half completes
    early.
  - Phase 1 (per core): Htab = feat @ W_fc for all 40064 padded nodes
    (PE), PSUM evacuated by a cost-balanced mix of Pool/ACT/DVE copies,
    streamed to DRAM on SP/ACT/Pool queues.  QT tiles computed first and
    kept in SBUF.  K^T gathers for the first groups run on Pool from
    t~0 (featR needs no compute).
  - Phase 2: edges in groups of GPB=2 blocks.  Per group, Pool issues
    transposed gathers of K^T [c,j] from featR (elem 256B) and int64-
    viewed row gathers of h (32 elems/idx) from Htab, sharing one int16
    index array.  Per 128-edge tile, PE computes all-pairs scores
    S^T[j,d] = K^T.T @ QT into PSUM, then adds 256*onehot(dst) with a
    single fp8 DoubleRow matmul (lhsT = host-built one-hot in [64,2,128]
    layout, rhs = 256*I in the matching layout; 0.5 cycles/row).  One
    ACT Exp over an 8-tile PSUM region yields the masked softmax
    numerators M2[j,d] = exp(e_j - c0) * onehot (mismatches underflow:
    exp(-256/sqrt(128)) ~ 1.5e-10).  Two more PE matmuls accumulate
    U[d,:] += M2^T @ h and denom[d] += M2^T @ 1.  Block epilogue (DVE):
    reciprocal + scaled copy, then DMA the rows out on SP.
  - exp shift c0 >= max e is host-computed, so no segment-max pass.
  - The host unpermutes the output rows and zeroes deg-0 nodes.
"""

import sys

sys.path.insert(0, "/opt/trn_rl_repo")

import numpy as np

import concourse.bacc as bacc
import concourse.mybir as mybir
import concourse.tile as tile
from concourse.bass_utils import run_bass_kernel_spmd
from concourse.library_config import mlp as mlp_lib

dt = mybir.dt
BF16 = dt.np(dt.bfloat16)
FP8 = dt.np(dt.float8e4)

N = 40000
E = 640000
F = 128
CORES = 8
NPC = N // CORES            # 5000 dst nodes per core
BLK = 128                   # dst nodes per block
NBLK = (NPC + BLK - 1) // BLK   # 40 blocks per core
SPLIT = 22528               # low/high table split; both halves < 2**15 rows
NPADT = 40064               # node count padded to 128 multiple (313 tiles)
NT_GLOBAL = NPADT // 128    # 313
SCALE = float(np.sqrt(np.float32(F)))
BIGSCALE = 224.0            # exactly representable in fp8e4 (max 240)
GPB = 2                     # blocks per gather group
NG = NBLK // GPB            # groups per core
OCT = 8                     # tiles per ACT exp call
PFK = 5                     # K-gather groups prefetched from t=0
USE_DR = True               # fp8 DoubleRow mask matmul (0.5 cycles/row)
USE_I64 = False               # int64-viewed h gathers (32 elems/idx)


def _pack_nodes(deg_low, deg_high):
    """Assign nodes to CORES*NBLK bins (<=128 nodes each), balancing the
    per-bin low/high edge counts to minimize gather padding."""
    import heapq
    nbins = CORES * NBLK
    nodes = np.argsort(-(deg_low + deg_high), kind="stable")
    # caps target whole tile counts: ceil(avg/128) tiles per (bin, half)
    cap_l = max(np.ceil(float(deg_low.sum()) / nbins / 128) * 128 - 2.0, 1.0)
    cap_h = max(np.ceil(float(deg_high.sum()) / nbins / 128) * 128 - 2.0, 1.0)
    bin_low = np.zeros(nbins, np.int64)
    bin_high = np.zeros(nbins, np.int64)
    bin_n = np.zeros(nbins, np.int64)
    node_bin = np.zeros(N, np.int64)
    node_slot = np.zeros(N, np.int64)
    heap = [(0.0, b) for b in range(nbins)]
    heapq.heapify(heap)
    for n in nodes:
        while True:
            k, b = heapq.heappop(heap)
            cur = max(bin_low[b] / cap_l, bin_high[b] / cap_h)
            if bin_n[b] >= 128:
                continue
            if k < cur - 1e-12:         # stale key: reinsert
                heapq.heappush(heap, (cur, b))
                continue
            break
        node_bin[n] = b
        node_slot[n] = bin_n[b]
        bin_n[b] += 1
        bin_low[b] += deg_low[n]
        bin_high[b] += deg_high[n]
        if bin_n[b] < 128:
            heapq.heappush(
                heap, (max(bin_low[b] / cap_l, bin_high[b] / cap_h), b))
    return node_bin, node_slot


def _host_prep(feat, W_fc, Wq, Wk, src, dst):
    """Shard edges by dst into (core, group, src-half, parity) gather calls
    with uniform padding.  Returns T_low/T_high, the shared gather index
    array, the fp8 DoubleRow one-hot mask M1x, node permutation, c0, deg."""
    half = (src >= SPLIT).astype(np.int64)
    deg_low = np.bincount(dst[half == 0], minlength=N)
    deg_high = np.bincount(dst[half == 1], minlength=N)
    node_bin, node_slot = _pack_nodes(deg_low, deg_high)

    bin_of = node_bin[dst]
    blk_of = bin_of % NBLK
    counts_bh = np.bincount(bin_of * 2 + half, minlength=CORES * NBLK * 2)
    T_low = int(np.ceil(counts_bh[0::2].max() / 128))
    T_high = int(np.ceil(counts_bh[1::2].max() / 128))
    T_blk = T_low + T_high
    GT = GPB * T_blk
    ntiles = NG * GT

    g_of = blk_of // GPB
    par_of = blk_of % GPB
    core_of = bin_of // NBLK
    gkey = ((core_of * NG + g_of) * 2 + half) * GPB + par_of
    nkeys = CORES * NG * 2 * GPB
    counts = np.bincount(gkey, minlength=nkeys)

    order = np.argsort(gkey, kind="stable")
    gk_s = gkey[order]
    src_s = src[order]
    drel_s = node_slot[dst][order]

    starts = np.zeros(nkeys + 1, np.int64)
    np.cumsum(counts, out=starts[1:])
    pos = np.arange(E, dtype=np.int64) - starts[gk_s]

    ks = np.arange(nkeys)
    k_g = (ks // (2 * GPB)) % NG
    k_half = (ks // GPB) % 2
    k_par = ks % GPB
    k_tile_base = k_g * GT + np.where(
        k_half == 0, k_par * T_low, GPB * T_low + k_par * T_high)

    slot = k_tile_base[gk_s] * 128 + pos
    lane = slot % 128
    tl = slot // 128
    core_s = gk_s // (NG * 2 * GPB)

    # shared gather indices (16-row wrap, tiled to 128 partitions)
    ncols = ntiles * 8
    idx_val = np.where(gk_s % (2 * GPB) < GPB, src_s, src_s - SPLIT) \
        .astype(np.int16)
    idx16 = np.zeros((CORES, 16, ncols), np.int16)
    col = k_tile_base[gk_s] * 8 + pos // 16
    row = pos % 16
    idx16[core_s, row, col] = idx_val
    idx16 = np.tile(idx16, (1, 8, 1))

    # fp8 one-hot mask in DoubleRow layout: tile tl occupies partitions
    # (tl%2)*64..(tl%2)*64+64, cols (tl//2)*256 + ihalf*128 + lane, where
    # slot s = ihalf*64 + krow.  1 where dstrel == s (pad cols all-zero).
    m1 = np.zeros((CORES, 128, (ntiles // 2) * 256), FP8)
    krow = drel_s % 64
    ihalf = drel_s // 64
    m1[core_s, (tl % 2) * 64 + krow, (tl // 2) * 256 + ihalf * 128 + lane] \
        = np.float32(1.0)

    perm = np.full((CORES * NBLK, 128), -1, np.int64)
    perm[node_bin, node_slot] = np.arange(N)

    # softmax shift: any constant >= max(e) keeps exp in range
    Qh = feat @ Wq
    Kh = feat @ Wk
    emax = -np.inf
    for i in range(0, E, 131072):
        sl = slice(i, min(i + 131072, E))
        e = np.einsum("ij,ij->i", Qh[dst[sl]], Kh[src[sl]]) / SCALE
        emax = max(emax, float(e.max()))
    c0 = float(emax)

    deg = deg_low + deg_high
    return T_low, T_high, idx16, m1, perm, c0, deg


def _build_program(T_low, T_high, c0):
    T_blk = T_low + T_high
    GT = GPB * T_blk
    ntiles = NG * GT
    ncols = ntiles * 8

    nc = bacc.Bacc("TRN2", target_bir_lowering=False, debug=False,
                   num_devices=CORES)

    featT_d = nc.dram_tensor("featT", [128, NPADT], dt.bfloat16,
                             kind="ExternalInput")
    featR_d = nc.dram_tensor("featR", [NPADT, 128], dt.bfloat16,
                             kind="ExternalInput")
    featTq_d = nc.dram_tensor("featTq", [128, NBLK * 128], dt.bfloat16,
                              kind="ExternalInput")
    Wfc_d = nc.dram_tensor("Wfc", [128, 128], dt.bfloat16, kind="ExternalInput")
    B_d = nc.dram_tensor("B", [128, 128], dt.bfloat16, kind="ExternalInput")
    gidx_d = nc.dram_tensor("gidx", [128, ncols], dt.int16, kind="ExternalInput")
    M1x_d = nc.dram_tensor("M1x", [128, (ntiles // 2) * 256], dt.float8e4,
                           kind="ExternalInput")
    IBS_d = nc.dram_tensor("IBS", [128, 256], dt.float8e4, kind="ExternalInput")
    bias_d = nc.dram_tensor("bias", [128, 1], dt.float32, kind="ExternalInput")
    Htab_d = nc.dram_tensor("Htab", [NPADT, 128], dt.bfloat16)
    rst_d = nc.dram_tensor("rst", [NBLK * BLK, 128], dt.float32,
                           kind="ExternalOutput")
    if USE_I64:
        Htab8 = Htab_d.ap().bitcast(dt.int64)  # [NPADT, 32] i64 view
        HELEM = 32
        HDT = dt.int64
    else:
        Htab8 = Htab_d.ap().bitcast(dt.float32)
        HELEM = 64
        HDT = dt.float32

    with tile.TileContext(nc) as tc:
        nc.gpsimd.load_library(mlp_lib)
        with tc.tile_pool(name="const", bufs=1) as cp:
            # gidx first: the t=0 K gathers need it
            gidx_sb = cp.tile([128, ncols], dt.int16, tag="gidx")
            nc.sync.dma_start(out=gidx_sb[:], in_=gidx_d.ap())
            # one-hot rhs duplicated across both partition halves so odd
            # tiles (lhsT at base partition 64) have a matching-base rhs
            IBS_sb = cp.tile([128, 2, 128], dt.float8e4, tag="ibs")
            nc.sync.dma_start(out=IBS_sb[:],
                              in_=IBS_d.ap().rearrange("p (i c) -> p i c", i=2))
            bias_sb = cp.tile([128, 1], dt.float32, tag="bias")
            nc.sync.dma_start(out=bias_sb[:], in_=bias_d.ap())
            ones_sb = cp.tile([128, 1], dt.bfloat16, tag="ones")
            nc.vector.memset(ones_sb[:], 1.0)
            Wfc_sb = cp.tile([128, 128], dt.bfloat16, tag="wfc")
            nc.sync.dma_start(out=Wfc_sb[:], in_=Wfc_d.ap())
            B_sb = cp.tile([128, 128], dt.bfloat16, tag="bmat")
            nc.sync.dma_start(out=B_sb[:], in_=B_d.ap())
            QT_sb = cp.tile([128, NBLK, 128], dt.bfloat16, tag="qt")

            # phase-2 SBUF pools open FIRST so their addresses do not
            # overlap phase-1's big tiles (else the prefetched gathers wait
            # on the phase-1 pool release)
            import contextlib
            with contextlib.ExitStack() as p2stack:
                gkt = p2stack.enter_context(tc.tile_pool(name="gkt", bufs=PFK + 1))
                ghb = p2stack.enter_context(tc.tile_pool(name="ghb", bufs=3))
                m1p = p2stack.enter_context(tc.tile_pool(name="m1p", bufs=2))
                m2p = p2stack.enter_context(tc.tile_pool(name="m2p", bufs=3))
                epp = p2stack.enter_context(tc.tile_pool(name="ep", bufs=4))

                nLt = GPB * T_low               # low tiles per group
                nL = nLt * 128
                nH = GPB * T_high * 128

                KT = [None] * NG

                def _gatherK(g):
                    """Both-halves transposed K gathers from featR (no
                    table dependency — runs as soon as gidx is loaded)."""
                    cb = g * GT * 8
                    KT[g] = gkt.tile([128, 1, GT * 128], dt.bfloat16,
                                     tag="kt", name=f"kt_{g}")
                    nc.gpsimd.dma_gather(
                        out_ap=KT[g][:, :, 0:nL],
                        in_ap=featR_d.ap()[0:SPLIT, :],
                        idxs_ap=gidx_sb[:, cb:cb + nL // 16],
                        num_idxs=nL, num_idxs_reg=nL,
                        elem_size=128, elem_step=128,
                        transpose=True, single_packet=False)
                    nc.gpsimd.dma_gather(
                        out_ap=KT[g][:, :, nL:GT * 128],
                        in_ap=featR_d.ap()[SPLIT:NPADT, :],
                        idxs_ap=gidx_sb[:, cb + nL // 16:cb + GT * 8],
                        num_idxs=nH, num_idxs_reg=nH,
                        elem_size=128, elem_step=128,
                        transpose=True, single_packet=False)

                # K prefetch for the first PFK groups: issued before any
                # phase-1 work so Pool streams them from t~0.
                for g0 in range(PFK):
                    _gatherK(g0)

                # ---- phase 1: QT tiles (SBUF) then Htab (DRAM) ----
                with tc.tile_pool(name="p1big", bufs=1) as p1big, \
                     tc.tile_pool(name="p1", bufs=5) as p1, \
                     tc.tile_pool(name="p1p", bufs=4, space="PSUM") as p1p, \
                     tc.tile_pool(name="p1q", bufs=2, space="PSUM") as p1q:
                    featTq_sb = p1big.tile([128, NBLK * 128], dt.bfloat16,
                                           tag="featTq")
                    nc.sync.dma_start(out=featTq_sb[:], in_=featTq_d.ap())
                    featT_sb = p1big.tile([128, NPADT], dt.bfloat16, tag="featT")
                    CH = NPADT // 8
                    for ci in range(8):
                        eng = nc.sync if ci < 6 else nc.scalar
                        eng.dma_start(
                            out=featT_sb[:, ci * CH:(ci + 1) * CH],
                            in_=featT_d.ap()[:, ci * CH:(ci + 1) * CH])

                    # QT tiles first (only featTq + B needed): PE warm-up
                    for b in range(NBLK):
                        psq = p1q.tile([128, 128], dt.float32, tag="qp")
                        nc.tensor.matmul(
                            psq[:], lhsT=B_sb[:],
                            rhs=featTq_sb[:, 128 * b:128 * (b + 1)],
                            start=True, stop=True)
                        if b % 2 == 0:
                            nc.scalar.activation(QT_sb[:, b, :], psq[:],
                                                 mybir.ActivationFunctionType.Copy)
                        else:
                            nc.vector.tensor_copy(out=QT_sb[:, b, :], in_=psq[:])

                    GRP = 8
                    # evacuation engines greedily balanced by modeled cost
                    evP, evA, evD = 0.0, 0.0, 1e-9
                    for g0 in range(0, NT_GLOBAL, GRP):
                        gn = min(GRP, NT_GLOBAL - g0)
                        ev = p1.tile([128, GRP, 128], dt.bfloat16, tag="he")
                        for q0 in range(0, gn, 4):
                            qn = min(4, gn - q0)
                            ps = p1p.tile([128, 4, 128], dt.float32, tag="hp")
                            for qi in range(qn):
                                g = g0 + q0 + qi
                                nc.tensor.matmul(
                                    ps[:, qi, :],
                                    lhsT=featT_sb[:, 128 * g:128 * (g + 1)],
                                    rhs=Wfc_sb[:], start=True, stop=True)
                            # GPSIMD cannot read PSUM (BIR verifier), so
                            # evacuate on ACT (143ns/t) / DVE (165ns/t) only
                            picks = [(evA + qn * 143, 'A'), (evD + qn * 165, 'D')]
                            picks.sort()
                            _, who = picks[0]
                            if who == 'A':
                                nc.scalar.activation(
                                    ev[:, q0:q0 + qn, :], ps[:, 0:qn, :],
                                    mybir.ActivationFunctionType.Copy)
                                evA += qn * 143
                            else:
                                nc.vector.tensor_copy(out=ev[:, q0:q0 + qn, :],
                                                      in_=ps[:, 0:qn, :])
                                evD += qn * 165
                        r0 = 128 * g0
                        r1 = 128 * (g0 + gn)
                        out_ap = Htab_d.ap()[r0:r1, :] \
                            .rearrange("(t p) c -> p t c", p=128)
                        wq = (g0 // GRP) % 3
                        weng = nc.sync if wq == 0 else (
                            nc.scalar if wq == 1 else nc.gpsimd)
                        weng.dma_start(out=out_ap, in_=ev[:, 0:gn, :])

                # ---- phase 2: edges, one GPB-block group at a time ----
                n_oct = (GT + OCT - 1) // OCT
                with tc.tile_pool(name="spsu", bufs=2, space="PSUM") as spsu, \
                     tc.tile_pool(name="upsu", bufs=1, space="PSUM") as upsu:
                    M1pre = [None, None]
                    for gp0 in range(2):
                        M1pre[gp0] = m1p.tile([128, (GT // 2) * 256],
                                              dt.float8e4, tag="m1",
                                              name=f"m1pre_{gp0}")
                        nc.sync.dma_start(
                            out=M1pre[gp0][:],
                            in_=M1x_d.ap()[:, gp0 * (GT // 2) * 256:
                                           (gp0 + 1) * (GT // 2) * 256])
                    for g in range(NG):
                        if g + PFK < NG:
                            _gatherK(g + PFK)
                        cb = g * GT * 8
                        # h row gathers (int64 view: 32 elems/idx)
                        HBL = ghb.tile([128, nLt, HELEM], HDT, tag="hbl")
                        nc.gpsimd.dma_gather(
                            out_ap=HBL[:, :, :],
                            in_ap=Htab8[0:SPLIT, :],
                            idxs_ap=gidx_sb[:, cb:cb + nL // 16],
                            num_idxs=nL, num_idxs_reg=nL,
                            elem_size=HELEM, elem_step=HELEM,
                            single_packet=False)
                        HBH = ghb.tile([128, GPB * T_high, HELEM], HDT,
                                       tag="hbh")
                        nc.gpsimd.dma_gather(
                            out_ap=HBH[:, :, :],
                            in_ap=Htab8[SPLIT:NPADT, :],
                            idxs_ap=gidx_sb[:, cb + nL // 16:cb + GT * 8],
                            num_idxs=nH, num_idxs_reg=nH,
                            elem_size=HELEM, elem_step=HELEM,
                            single_packet=False)
                        if g < 2:
                            M1 = M1pre[g]
                        else:
                            M1 = m1p.tile([128, (GT // 2) * 256],
                                          dt.float8e4, tag="m1")
                            nc.sync.dma_start(
                                out=M1[:],
                                in_=M1x_d.ap()[:, g * (GT // 2) * 256:
                                               (g + 1) * (GT // 2) * 256])
                        KTg = KT[g]
                        M1v = M1.rearrange("p (t i c) -> p t i c", i=2, c=128)
                        HBLb = HBL.bitcast(dt.bfloat16)   # [128, nLt, 128]
                        HBHb = HBH.bitcast(dt.bfloat16)

                        def _parity(t):
                            if t < nLt:
                                return t // T_low
                            return (t - nLt) // T_high

                        def _kt(t):
                            return KTg[:, 0, t * 128:(t + 1) * 128]

                        def _hb(t):
                            if t < nLt:
                                return HBLb[:, t, :]
                            return HBHb[:, t - nLt, :]

                        U = [upsu.tile([128, 128], dt.float32, space="PSUM",
                                       tag=f"U{p}", name=f"U{p}_{g}")
                             for p in range(GPB)]
                        Dn = [upsu.tile([128, 8], dt.float32, space="PSUM",
                                        tag=f"Dn{p}", name=f"Dn{p}_{g}")
                              for p in range(GPB)]

                        S = [None] * n_oct
                        M2 = [None] * n_oct

                        def _mm3(o):
                            t0 = o * OCT
                            for ti in range(min(OCT, GT - t0)):
                                t = t0 + ti
                                p = _parity(t)
                                first = (t == p * T_low)
                                last = (t == nLt + (p + 1) * T_high - 1)
                                nc.tensor.matmul(U[p][:, :], lhsT=M2[o][:, ti, :],
                                                 rhs=_hb(t),
                                                 start=first, stop=last)
                                nc.tensor.matmul(Dn[p][:, 0:1],
                                                 lhsT=M2[o][:, ti, :],
                                                 rhs=ones_sb[:],
                                                 start=first, stop=last)

                        for o in range(n_oct):
                            t0 = o * OCT
                            on = min(OCT, GT - t0)
                            S[o] = spsu.tile([128, OCT, 128], dt.float32,
                                             space="PSUM", tag="soct",
                                             name=f"soct_{g}_{o}")
                            for ti in range(on):
                                t = t0 + ti
                                nc.tensor.matmul(
                                    S[o][:, ti, :],
                                    lhsT=_kt(t),
                                    rhs=QT_sb[:, GPB * g + _parity(t), :],
                                    start=True, stop=False)
                                hp = (t % 2) * 64
                                if USE_DR:
                                    nc.tensor.matmul(
                                        S[o][:, ti, :],
                                        lhsT=M1v[hp:hp + 64, t // 2, :, :],
                                        rhs=IBS_sb[hp:hp + 64, :, :],
                                        start=False, stop=True,
                                        perf_mode=mybir.MatmulPerfMode.DoubleRow)
                                else:
                                    for ih in range(2):
                                        nc.tensor.matmul(
                                            S[o][:, ti, :],
                                            lhsT=M1v[hp:hp + 64, t // 2, ih, :],
                                            rhs=IBS_sb[hp:hp + 64, ih, :],
                                            start=False, stop=(ih == 1))
                            M2[o] = m2p.tile([128, OCT, 128], dt.bfloat16,
                                             tag="m2oct", name=f"m2oct_{g}_{o}")
                            nc.scalar.activation(M2[o][:, 0:on, :], S[o][:, 0:on, :],
                                                 mybir.ActivationFunctionType.Exp,
                                                 bias=bias_sb[:, 0:1],
                                                 scale=1.0 / SCALE)
                            if o >= 1:
                                _mm3(o - 1)
                        _mm3(n_oct - 1)

                        for p in range(GPB):
                            b = GPB * g + p
                            dg = epp.tile([128, 1], dt.float32, tag="dg")
                            nc.vector.tensor_scalar(
                                out=dg[:], in0=Dn[p][:, 0:1], scalar1=1e-30,
                                scalar2=None, op0=mybir.AluOpType.add)
                            rr = epp.tile([128, 1], dt.float32, tag="rr")
                            nc.vector.reciprocal(rr[:], dg[:])
                            ro = epp.tile([128, 128], dt.float32, tag="ro")
                            nc.vector.tensor_scalar(
                                out=ro[:], in0=U[p][:, :], scalar1=rr[:, 0:1],
                                scalar2=None, op0=mybir.AluOpType.mult)
                            nc.sync.dma_start(
                                out=rst_d.ap()[b * BLK:(b + 1) * BLK, :],
                                in_=ro[:])
    nc.finalize()
    return nc


def _make_in_maps(feat, W_fc, Wq, Wk, idx16, m1, perm, c0):
    featT = feat.T.astype(BF16)
    featT_pad = np.zeros((128, NPADT), BF16)
    featT_pad[:, :N] = featT
    featR_pad = np.zeros((NPADT, 128), BF16)
    featR_pad[:N, :] = feat.astype(BF16)
    Bm = (Wq @ Wk.T).astype(BF16)
    Wfc_b = W_fc.astype(BF16)
    # IBS DoubleRow rhs: [64, 2, 128] flattened, 256*delta(i*64+k == d)
    IBS = np.zeros((64, 2, 128), np.float32)
    for i in range(2):
        IBS[np.arange(64), i, i * 64 + np.arange(64)] = BIGSCALE
    IBS = np.tile(IBS.reshape(64, 256), (2, 1)).astype(FP8)
    bias = np.full((128, 1), -c0 - BIGSCALE / SCALE, np.float32)

    in_maps = []
    for c in range(CORES):
        pc = perm[c * NBLK:(c + 1) * NBLK].reshape(-1)
        fq = np.zeros((128, NBLK * 128), BF16)
        valid = pc >= 0
        fq[:, valid] = featT[:, pc[valid]]
        in_maps.append({
            "featT": featT_pad,
            "featR": featR_pad,
            "featTq": fq,
            "Wfc": Wfc_b,
            "B": Bm,
            "gidx": np.ascontiguousarray(idx16[c]),
            "M1x": np.ascontiguousarray(m1[c]),
            "IBS": IBS,
            "bias": bias,
        })
    return in_maps


_CACHE = {}


def kernel(feat, loc, W_fc, Wq, Wk, Wq2, Wk2, G_w, embed, boundaries,
           src, dst, inter_ids, **_ignored):
    feat = np.asarray(feat, np.float32)
    W_fc = np.asarray(W_fc, np.float32)
    Wq = np.asarray(Wq, np.float32)
    Wk = np.asarray(Wk, np.float32)
    src = np.asarray(src).astype(np.int64)
    dst = np.asarray(dst).astype(np.int64)

    T_low, T_high, idx16, m1, perm, c0, deg = _host_prep(
        feat, W_fc, Wq, Wk, src, dst)

    key = (T_low, T_high, round(c0, 4))
    if key not in _CACHE:
        _CACHE[key] = _build_program(T_low, T_high, c0)
    nc = _CACHE[key]

    in_maps = _make_in_maps(feat, W_fc, Wq, Wk, idx16, m1, perm, c0)

    res = run_bass_kernel_spmd(nc, in_maps, core_ids=list(range(CORES)))
    out = np.zeros((N, F), np.float32)
    for c in range(CORES):
        pc = perm[c * NBLK:(c + 1) * NBLK].reshape(-1)
        valid = pc >= 0
        out[pc[valid]] = res.results[c]["rst"][valid]
    out[deg == 0] = 0.0
    return out
